# revision 22
# baseline (speedup 1.0000x reference)
"""Trainium2 Bass kernel for nn_KernelEncoderLayer (gnn_message_passing).

Math (per graph b of 4, N=1024 points, K=9 kernel offsets, C=32 channels):
  y[i,c] = leaky( sum_{n,k} exp(-|pi - pn - kk|^2/2) * (w @ conv_w[k])[n,c] )
  out = BN(y)+w -> MLP(32->128->32, leaky, BN) residual.

Factorization (k-independent Gaussian Gram matrix):
  exp(-|pi - pn - kk|^2/2) = G[n,i] * A[i,k] * B[n,k]
  G[n,i] = exp(pn.pi - |pn|^2/2 - |pi|^2/2)
  A[i,k] = exp(pi.kk),  B[n,k] = exp(-pn.kk - |kk|^2/2)   (host-precomputed)
So y[i,c] = sum_k A[i,k] * (G.T @ (B[:,k] * cw[:,k,:]))[i,c].

v2 performance structure:
  - G's pn.pi matmul runs in bf16 with a hi/lo split of the coordinates
    (contract dim 8) -> full tensor-engine rate with ~fp32 exponent accuracy.
  - Phase B / phase D matmuls in bf16 (1 cyc/row); MLP matmuls in fp32r.
  - B and A factors are host-replicated across the 32 channels so the
    (n,k)- and (i,k)-scalings are a few big vector ops, not 100+ tiny ones.
  - ONE AllGather ships each core's yT slice + BN0 partial sums; every core
    then redundantly computes full-batch h (for BN1 stats) and its own
    output slice fully locally (no second collective, no rank-dependent
    indexing).
  - BN rsqrt via DVE Newton iteration (quake trick) -> no Sqrt tables on
    the scalar engine; ACT only ever loads Exp and Lrelu tables.
  - Optional tiny dummy AllGather at kernel start absorbs first-collective
    warmup / rank-start skew concurrently with compute.

Sharding: 8 cores = 4 graphs x 2 halves of the 1024 output rows. Each core
computes its [512, 32] conv-output slice and emits out[32, 512]; the host
concatenates.

Self-contained: hardcodes B=4, N=1024, K=9, C=32, CM=128, 8 cores.
"""

import numpy as np
import ml_dtypes

import concourse.bass as bass
import concourse.bacc as bacc
import concourse.mybir as mybir
import concourse.tile as tile
from concourse import masks
from concourse.bass_utils import run_bass_kernel_spmd

F32 = mybir.dt.float32
F32R = mybir.dt.float32r
BF16 = mybir.dt.bfloat16
I32 = mybir.dt.int32
AF = mybir.ActivationFunctionType
ALU = mybir.AluOpType
AX = mybir.AxisListType

NB, N, K, C, CM = 4, 1024, 9, 32, 128
NCORES = 8
EPS = 1e-5
SLOPE = 0.01
NT = NB * N
KC = K * C  # 288
MAGIC = 0x5F3759DF

WARM_CC = True  # early dummy collective to absorb first-CC warmup


def _r(ap):
    return ap.bitcast(F32R)


def _build_module():
    nc = bacc.Bacc("TRN2", target_bir_lowering=False, debug=False,
                   num_devices=NCORES)

    def din(name, shape, dt=F32):
        return nc.dram_tensor(name, list(shape), dt, kind="ExternalInput").ap()

    pn8_d = din("pn8", (8, N), BF16)        # hi/lo split coords, n side
    pi8_d = din("pi8", (8, 512), BF16)      # hi/lo split coords + bias rows
    p128_d = din("p128", (128, 44))         # negsqn(8)|w2(32)|g1|be1|i32 consts
    wTb_d = din("wTb", (C, N), BF16)        # graph weights transposed, bf16
    cwall_d = din("cwall", (C, KC), BF16)   # conv_w as [d, k*32+c]
    bexp_d = din("bexp", (128, 8 * KC))     # B[n,k] replicated over c
    aexp_d = din("aexp", (128, 4 * KC))     # A[i,k] replicated over c
    wtext_d = din("wtext", (C, NT), BF16)   # full-batch weights transposed
    w1b16_d = din("w1b16", (C, CM), BF16)   # w1 in bf16 for MLP1 matmuls
    wtob_d = din("wtob", (C, 512), BF16)    # own-slice weights bf16
    w1e_d = din("w1e", (C + 1, CM))         # w1 rows + b1 row
    p32_d = din("p32", (C, 3))              # g0 | be0 | b2
    wto_d = din("wto", (C, 512))            # own-slice weights transposed

    out_d = nc.dram_tensor("out", [C, 512], F32, kind="ExternalOutput").ap()
    dbg_y_d = nc.dram_tensor("dbg_y", [C, 516], BF16,
                             kind="ExternalOutput").ap()
    dbg_s_d = nc.dram_tensor("dbg_s", [128, 8], F32,
                             kind="ExternalOutput").ap()

    with tile.TileContext(nc) as tc:
        with (
            tc.tile_pool(name="const", bufs=1) as pc,
            tc.tile_pool(name="big", bufs=1) as pb,
            tc.tile_pool(name="work", bufs=3) as pw,
            tc.tile_pool(name="dram", bufs=1, space="DRAM") as pd,
        ):
            # ---- optional collective warmup (content irrelevant) ----
            if WARM_CC:
                dmy_in = pd.tile([1, 8], F32, tag="dmy_in")
                dmy_out = pd.tile([NCORES, 8], F32, tag="dmy_out")
                nc.gpsimd.collective_compute(
                    "AllGather", ALU.bypass,
                    replica_groups=[list(range(NCORES))],
                    ins=[dmy_in[:].opt()], outs=[dmy_out[:].opt()])

            # ---- input loads ----
            def load(name, ap, shape, dt=F32, pool=pc):
                t = pool.tile(list(shape), dt, tag=name, name=name)
                nc.sync.dma_start(out=t[:], in_=ap)
                return t

            pn8 = load("pn8", pn8_d, (8, N), BF16)
            pi8 = load("pi8", pi8_d, (8, 512), BF16)
            p128 = load("p128", p128_d, (128, 44))
            wTb = load("wTb", wTb_d, (C, N), BF16)
            cwall = load("cwall", cwall_d, (C, KC), BF16)
            bexp = load("bexp", bexp_d, (128, 8 * KC))
            aexp = load("aexp", aexp_d, (128, 4 * KC))
            wtext = load("wtext", wtext_d, (C, NT), BF16, pool=pb)
            w1b16 = load("w1b16", w1b16_d, (C, CM), BF16)
            wtob = load("wtob", wtob_d, (C, 512), BF16)
            w1e = load("w1e", w1e_d, (C + 1, CM))
            p32 = load("p32", p32_d, (C, 3))
            wto = load("wto", wto_d, (C, 512))

            negsqn = p128[:, 0:8]
            w2 = p128[:, 8:40]
            g1c = p128[:, 40:41]
            be1c = p128[:, 41:42]
            g0c = p32[:, 0:1]
            be0c = p32[:, 1:2]
            b2c = p32[:, 2:3]

            ident = pc.tile([128, 128], F32, tag="ident")
            masks.make_identity(nc, ident[:])

            ag_y = pb.tile([C, 516], BF16, tag="ag_y")
            ysum_p = pc.tile([C, 4], F32, tag="ysum")
            ysq_p = pc.tile([C, 4], F32, tag="ysq")

            with (
                tc.tile_pool(name="psG", bufs=2, space="PSUM") as psG,
                tc.tile_pool(name="psB", bufs=2, space="PSUM") as psB,
                tc.tile_pool(name="psD", bufs=2, space="PSUM") as psD,
                tc.tile_pool(name="psC", bufs=1, space="PSUM") as psC,
            ):
                # ---- G[n,i] = exp(pn.pi - |pn|^2/2 - |pi|^2/2), bf16 hi/lo
                g_sb = [pb.tile([128, 512], BF16, tag=f"g{j}", name=f"g{j}")
                        for j in range(8)]
                # ---- cw'[n,(k,c)] = B[n,k] * (w @ conv_w[k]), bf16
                cw_sb = [pb.tile([128, KC], BF16, tag=f"cw{j}", name=f"cw{j}")
                         for j in range(8)]
                for j in range(8):
                    psg = psG.tile([128, 512], F32, tag="g")
                    nc.tensor.matmul(psg[:], lhsT=pn8[:, j * 128:(j + 1) * 128],
                                     rhs=pi8[:], start=True, stop=True)
                    nc.scalar.activation(g_sb[j][:], psg[:], AF.Exp,
                                         bias=negsqn[:, j:j + 1], scale=1.0)
                    psb = psB.tile([128, KC], F32, tag="b")
                    nc.tensor.matmul(psb[:], lhsT=wTb[:, j * 128:(j + 1) * 128],
                                     rhs=cwall[:], start=True, stop=True)
                    nc.vector.tensor_tensor(
                        cw_sb[j][:], psb[:],
                        bexp[:, j * KC:(j + 1) * KC], op=ALU.mult)

                # ---- main contraction + combine, two t's at a time ----
                # (2 PSUM accumulator banks; pair 0-1 combines while pair
                # 2-3 accumulates)
                for tp in range(2):
                    pys = [psD.tile([128, KC], F32, tag="py",
                                    name=f"py{tp}_{ti}")
                           for ti in range(2)]
                    for j in range(8):
                        for ti in range(2):
                            t = tp * 2 + ti
                            nc.tensor.matmul(
                                pys[ti][:],
                                lhsT=g_sb[j][:, t * 128:(t + 1) * 128],
                                rhs=cw_sb[j][:],
                                start=(j == 0), stop=(j == 7))
                    for ti in range(2):
                        t = tp * 2 + ti
                        ya = pw.tile([128, KC], F32, tag="ya")
                        nc.vector.tensor_tensor(
                            ya[:], pys[ti][:], aexp[:, t * KC:(t + 1) * KC],
                            op=ALU.mult)
                        y_t = pw.tile([128, C], F32, tag="yt")
                        nc.vector.tensor_reduce(
                            y_t[:], ya[:].rearrange("p (k c) -> p c k", k=K),
                            axis=AX.X, op=ALU.add)
                        y_l = pw.tile([128, C], F32, tag="yl")
                        nc.vector.scalar_tensor_tensor(
                            y_l[:], y_t[:], SLOPE, y_t[:],
                            op0=ALU.mult, op1=ALU.max)
                        ptr = psC.tile([C, 128], F32, tag="tr")
                        nc.tensor.transpose(ptr[:], y_l[:], ident[:])
                        nc.vector.tensor_scalar(
                            ag_y[:, t * 128:(t + 1) * 128], ptr[:], 0.0, 0.0,
                            op0=ALU.add, op1=ALU.add,
                            accum_out=ysum_p[:, t:t + 1])
                        agt = ag_y[:, t * 128:(t + 1) * 128]
                        sq = pw.tile([C, 128], BF16, tag="sq")
                        nc.vector.scalar_tensor_tensor(
                            sq[:], agt, 1.0, agt, op0=ALU.mult, op1=ALU.mult,
                            accum_out=ysq_p[:, t:t + 1])
                stat_cols = ag_y[:, 512:516].bitcast(F32)
                nc.vector.tensor_reduce(stat_cols[:, 0:1], ysum_p[:],
                                        axis=AX.X, op=ALU.add)
                nc.vector.tensor_reduce(stat_cols[:, 1:2], ysq_p[:],
                                        axis=AX.X, op=ALU.add)

            # ---- the one AllGather: own yT slice + BN0 partial sums ----
            ag_in = pd.tile([C, 516], BF16, tag="ag_in")
            ag_out = pd.tile([C * NCORES, 516], BF16, tag="ag_out")
            nc.sync.dma_start(out=ag_in[:], in_=ag_y[:])
            nc.gpsimd.collective_compute(
                "AllGather", ALU.bypass,
                replica_groups=[list(range(NCORES))],
                ins=[ag_in[:].opt()], outs=[ag_out[:].opt()])

            with (
                tc.tile_pool(name="psH", bufs=1, space="PSUM") as psH,
                tc.tile_pool(name="psR", bufs=2, space="PSUM") as psR,
                tc.tile_pool(name="psS", bufs=1, space="PSUM") as psS,
                tc.tile_pool(name="psT", bufs=1, space="PSUM") as psT,
            ):
                # ---- during the collective: w-side of MLP1 for chunks 0-2,
                # Lrelu table preload dummy
                NHOLD = 3
                hold = [psH.tile([128, 512], F32, tag=f"h{ci}", name=f"h{ci}")
                        for ci in range(NHOLD)]
                for ci in range(NHOLD):
                    nc.tensor.matmul(hold[ci][:], lhsT=w1b16[:],
                                     rhs=wtext[:, ci * 512:(ci + 1) * 512],
                                     start=True, stop=False)
                # preload the Sqrt table while the collective runs
                warm = pw.tile([128, 1], F32, tag="warm")
                nc.scalar.activation(warm[:], p128[:, 42:43], AF.Sqrt)

                # ---- receive the gather ----
                agv = ag_out[:].rearrange("(jj p) n -> p jj n", jj=NCORES)
                statj = pc.tile([C, 32], BF16, tag="statj")
                nc.sync.dma_start(
                    out=statj[:].rearrange("p (jj s) -> p jj s", s=4),
                    in_=agv[:, :, 512:516])
                yt_full = pb.tile([C, NT], BF16, tag="yt_full")
                nc.sync.dma_start(
                    out=yt_full[:, 0:2048].rearrange("p (jj n) -> p jj n",
                                                     jj=4),
                    in_=agv[:, 0:4, 0:512])
                nc.sync.dma_start(
                    out=yt_full[:, 2048:4096].rearrange("p (jj n) -> p jj n",
                                                        jj=4),
                    in_=agv[:, 4:8, 0:512])

                # ---- BN0 global stats ----
                statv = statj[:].bitcast(F32).rearrange("p (jj s) -> p s jj", s=2)
                tot = pc.tile([C, 2], F32, tag="tot")
                nc.vector.tensor_reduce(tot[:, 0:1], statv[:, 0:1, :],
                                        axis=AX.X, op=ALU.add)
                nc.vector.tensor_reduce(tot[:, 1:2], statv[:, 1:2, :],
                                        axis=AX.X, op=ALU.add)
                mom = pc.tile([C, 2], F32, tag="mom")
                nc.vector.tensor_scalar_mul(mom[:], tot[:], 1.0 / NT)
                var0 = pc.tile([C, 1], F32, tag="var0")
                nc.vector.tensor_tensor(var0[:], mom[:, 0:1], mom[:, 0:1],
                                        op=ALU.mult)
                nc.vector.scalar_tensor_tensor(
                    var0[:], mom[:, 1:2], EPS, var0[:],
                    op0=ALU.add, op1=ALU.subtract)

                def rsqrt(dst, src, p, pref):
                    # ACT Sqrt (table preloaded during the AG) + DVE recip
                    st = pc.tile([p, 1], F32, tag=pref + "s")
                    nc.scalar.activation(st[:], src[:], AF.Sqrt)
                    nc.vector.reciprocal(dst[:], st[:])

                rstd0 = pc.tile([C, 1], F32, tag="rstd0")
                rsqrt(rstd0, var0, C, "r0")
                scale0 = pc.tile([C, 1], F32, tag="scale0")
                nc.vector.tensor_tensor(scale0[:], rstd0[:], g0c,
                                        op=ALU.mult)
                nscale0 = pc.tile([C, 1], F32, tag="nscale0")
                nc.vector.tensor_scalar_mul(nscale0[:], scale0[:], -1.0)
                shift0 = pc.tile([C, 1], F32, tag="shift0")
                nc.vector.scalar_tensor_tensor(
                    shift0[:], mom[:, 0:1], nscale0[:], be0c,
                    op0=ALU.mult, op1=ALU.add)

                # bias1 = w1.T @ shift0 + b1  (ones-row trick via w1e row C)
                sh1e = pc.tile([C + 1, 1], F32, tag="sh1e")
                nc.vector.memset(sh1e[C:C + 1, :], 1.0)
                nc.vector.tensor_copy(out=sh1e[0:C, :], in_=shift0[:])
                psb1 = psT.tile([CM, 1], F32, tag="b1")
                nc.tensor.matmul(psb1[:], lhsT=w1e[:], rhs=sh1e[:],
                                 start=True, stop=True)
                bias1 = pc.tile([CM, 1], F32, tag="bias1")
                nc.vector.tensor_copy(out=bias1[:], in_=psb1[:])
                w1p = pc.tile([C, CM], BF16, tag="w1p")
                nc.vector.tensor_scalar(w1p[:], w1e[0:C, :], scale0[:], None,
                                        op0=ALU.mult)

                # ---- full-batch h chunks: leaky(w1'.T yT + w1.T wT + bias1)
                hsum_p = pc.tile([CM, 8], F32, tag="hsum")
                hsq_p = pc.tile([CM, 8], F32, tag="hsq")
                for ci in range(8):
                    if ci < NHOLD:
                        ps = hold[ci]
                    else:
                        ps = psR.tile([128, 512], F32, tag="hr")
                        nc.tensor.matmul(
                            ps[:], lhsT=w1b16[:],
                            rhs=wtext[:, ci * 512:(ci + 1) * 512],
                            start=True, stop=False)
                    nc.tensor.matmul(
                        ps[:], lhsT=w1p[:],
                        rhs=yt_full[:, ci * 512:(ci + 1) * 512],
                        start=False, stop=True)
                    hcs = pw.tile([CM, 512], BF16, tag="hcs")
                    nc.scalar.activation(hcs[:], ps[:], AF.Lrelu,
                                         bias=bias1[:], scale=1.0, alpha=SLOPE,
                                         accum_out=hsum_p[:, ci:ci + 1])
                    sqh = pw.tile([CM, 512], BF16, tag="sqh")
                    nc.vector.scalar_tensor_tensor(
                        sqh[:], hcs[:], 1.0, hcs[:],
                        op0=ALU.mult, op1=ALU.mult,
                        accum_out=hsq_p[:, ci:ci + 1])

                # ---- own-slice yres + h (fully local, no rank indexing) ----
                yres = pb.tile([C, 512], F32, tag="yres")
                nc.vector.tensor_scalar(yres[:], ag_y[:, 0:512], scale0[:],
                                        shift0[:], op0=ALU.mult, op1=ALU.add)
                nc.vector.tensor_tensor(yres[:], yres[:], wto[:], op=ALU.add)
                pso = psR.tile([128, 512], F32, tag="hr")
                nc.tensor.matmul(pso[:], lhsT=w1b16[:], rhs=wtob[:],
                                 start=True, stop=False)
                nc.tensor.matmul(pso[:], lhsT=w1p[:],
                                 rhs=ag_y[:, 0:512],
                                 start=False, stop=True)
                h_own = pb.tile([CM, 512], BF16, tag="h_own")
                nc.scalar.activation(h_own[:], pso[:], AF.Lrelu,
                                     bias=bias1[:], scale=1.0, alpha=SLOPE)

                # ---- BN1 stats + fold into w2 ----
                tot1 = pc.tile([CM, 2], F32, tag="tot1")
                nc.vector.tensor_reduce(tot1[:, 0:1], hsum_p[:], axis=AX.X,
                                        op=ALU.add)
                nc.vector.tensor_reduce(tot1[:, 1:2], hsq_p[:], axis=AX.X,
                                        op=ALU.add)
                mom1 = pc.tile([CM, 2], F32, tag="mom1")
                nc.vector.tensor_scalar_mul(mom1[:], tot1[:], 1.0 / NT)
                var1 = pc.tile([CM, 1], F32, tag="var1")
                nc.vector.tensor_tensor(var1[:], mom1[:, 0:1], mom1[:, 0:1],
                                        op=ALU.mult)
                nc.vector.scalar_tensor_tensor(
                    var1[:], mom1[:, 1:2], EPS, var1[:],
                    op0=ALU.add, op1=ALU.subtract)
                rstd1 = pc.tile([CM, 1], F32, tag="rstd1")
                rsqrt(rstd1, var1, CM, "r1")
                scale1 = pc.tile([CM, 1], F32, tag="scale1")
                nc.vector.tensor_tensor(scale1[:], rstd1[:], g1c,
                                        op=ALU.mult)
                nscale1 = pc.tile([CM, 1], F32, tag="nscale1")
                nc.vector.tensor_scalar_mul(nscale1[:], scale1[:], -1.0)
                shift1 = pc.tile([CM, 1], F32, tag="shift1")
                nc.vector.scalar_tensor_tensor(
                    shift1[:], mom1[:, 0:1], nscale1[:], be1c,
                    op0=ALU.mult, op1=ALU.add)
                w2p = pc.tile([CM, C], BF16, tag="w2p")
                nc.vector.tensor_scalar(w2p[:], w2, scale1[:], None,
                                        op0=ALU.mult)
                psb2 = psT.tile([C, 1], F32, tag="b2")
                nc.tensor.matmul(psb2[:], lhsT=w2, rhs=shift1[:],
                                 start=True, stop=True)
                bias2 = pc.tile([C, 1], F32, tag="bias2")
                nc.vector.tensor_tensor(bias2[:], psb2[:], b2c, op=ALU.add)

                # ---- delta = w2'.T @ h_own + bias2; out = yres + delta ----
                psd = psS.tile([C, 512], F32, tag="delta")
                nc.tensor.matmul(psd[:], lhsT=w2p[:], rhs=h_own[:],
                                 start=True, stop=True)
                out_sb = pw.tile([C, 512], F32, tag="outsb")
                nc.vector.scalar_tensor_tensor(
                    out_sb[:], psd[:], bias2[:], yres[:],
                    op0=ALU.add, op1=ALU.add)
                nc.sync.dma_start(out=out_d, in_=out_sb[:])

                # ---- debug dumps ----
                nc.sync.dma_start(out=dbg_y_d, in_=ag_y[:])
                dbg_s = pc.tile([128, 8], F32, tag="dbg_s")
                nc.vector.memset(dbg_s[:], 0.0)
                nc.vector.tensor_copy(out=dbg_s[0:C, 0:2], in_=tot[:])
                nc.vector.tensor_copy(out=dbg_s[0:C, 2:3], in_=var0[:])
                nc.vector.tensor_copy(out=dbg_s[0:C, 3:4], in_=rstd0[:])
                nc.vector.tensor_copy(out=dbg_s[0:C, 4:5], in_=scale0[:])
                nc.vector.tensor_copy(out=dbg_s[0:C, 5:6], in_=shift0[:])
                nc.vector.tensor_copy(out=dbg_s[:, 6:7], in_=bias1[:])
                nc.vector.tensor_copy(out=dbg_s[:, 7:8], in_=rstd1[:])
                nc.sync.dma_start(out=dbg_s_d, in_=dbg_s[:])

    nc.compile()
    return nc


_NC_CACHE = {}


def _get_module():
    if "nc" not in _NC_CACHE:
        _NC_CACHE["nc"] = _build_module()
    return _NC_CACHE["nc"]


BF = ml_dtypes.bfloat16


def _split_hi_lo(x):
    hi = x.astype(BF)
    lo = (x - hi.astype(np.float32)).astype(BF)
    return hi, lo


def _host_prep(inputs):
    pos = np.asarray(inputs["positions"], np.float32)
    w = np.asarray(inputs["weights"], np.float32)
    kp = np.asarray(inputs["kernel_pos"], np.float32)
    cw = np.asarray(inputs["conv_w"], np.float32)
    posb = pos.reshape(NB, N, 2)
    wb = w.reshape(NB, N, C)
    kk2 = 0.5 * (kp ** 2).sum(1)                       # [9]
    cwall = np.ascontiguousarray(
        cw.transpose(1, 0, 2).reshape(C, KC)).astype(BF)
    wTfull = np.ascontiguousarray(w.T)                 # [32, 4096]
    w1 = np.asarray(inputs["w1"], np.float32)
    w2 = np.asarray(inputs["w2"], np.float32)
    w1e = np.concatenate(
        [w1, np.asarray(inputs["b1"], np.float32).reshape(1, CM)], axis=0)
    p128 = np.zeros((128, 44), np.float32)
    p128[:, 8:40] = w2
    p128[:, 40] = np.asarray(inputs["bn1_gamma"], np.float32)
    p128[:, 41] = np.asarray(inputs["bn1_beta"], np.float32)
    p128[:, 42] = np.int32(1).view(np.float32)
    p128[:, 43] = np.int32(MAGIC).view(np.float32)
    p32 = np.stack([np.asarray(inputs["bn_gamma"], np.float32),
                    np.asarray(inputs["bn_beta"], np.float32),
                    np.asarray(inputs["b2"], np.float32)], axis=1)
    shared = dict(cwall=cwall, wtext=np.ascontiguousarray(wTfull).astype(BF),
                  w1b16=np.ascontiguousarray(w1).astype(BF),
                  w1e=np.ascontiguousarray(w1e), p32=np.ascontiguousarray(p32))

    in_maps = []
    for j in range(NCORES):
        b, off = j // 2, (j % 2) * 512
        p = posb[b]
        pi = p[off:off + 512]
        xh, xl = _split_hi_lo(p[:, 0])
        yh, yl = _split_hi_lo(p[:, 1])
        one = np.ones(N, BF)
        pn8 = np.stack([xh, xh, xl, yh, yh, yl, one, one])
        bias = -0.5 * (pi ** 2).sum(1)
        bh, bl = _split_hi_lo(bias)
        xih, xil = _split_hi_lo(pi[:, 0])
        yih, yil = _split_hi_lo(pi[:, 1])
        pi8 = np.stack([xih, xil, xih, yih, yil, yih, bh, bl])
        p128j = p128.copy()
        p128j[:, 0:8] = (-0.5 * (p ** 2).sum(1)).reshape(8, 128).T
        dotn = (p @ kp.T).astype(np.float32)            # [1024, 9]
        Bmat = np.exp(-dotn - kk2[None, :]).astype(np.float32)
        bexp = np.ascontiguousarray(np.broadcast_to(
            Bmat.reshape(8, 128, K, 1).transpose(1, 0, 2, 3),
            (128, 8, K, C)).reshape(128, 8 * KC))
        Amat = np.exp((pi @ kp.T).astype(np.float32)).astype(np.float32)
        aexp = np.ascontiguousarray(np.broadcast_to(
            Amat.reshape(4, 128, K, 1).transpose(1, 0, 2, 3),
            (128, 4, K, C)).reshape(128, 4 * KC))
        m = dict(pn8=np.ascontiguousarray(pn8),
                 pi8=np.ascontiguousarray(pi8),
                 p128=np.ascontiguousarray(p128j),
                 wTb=np.ascontiguousarray(wb[b].T).astype(BF),
                 bexp=bexp, aexp=aexp,
                 wto=np.ascontiguousarray(wTfull[:, j * 512:(j + 1) * 512]),
                 wtob=np.ascontiguousarray(
                     wTfull[:, j * 512:(j + 1) * 512]).astype(BF))
        m.update(shared)
        in_maps.append(m)
    return in_maps


def _run(inputs, trace=False):
    nc = _get_module()
    in_maps = _host_prep(inputs)
    res = run_bass_kernel_spmd(nc, in_maps, core_ids=list(range(NCORES)),
                               trace=trace)
    out = np.concatenate([np.asarray(res.results[j]["out"])
                          for j in range(NCORES)], axis=1)   # [32, 4096]
    return np.ascontiguousarray(out.T), res


def kernel(**inputs):
    out, _ = _run(inputs, trace=False)
    return out


# revision 25
# speedup vs baseline: 1.1221x; 1.1221x over previous
"""Trainium2 Bass kernel for nn_KernelEncoderLayer (gnn_message_passing).

Math (per graph b of 4, N=1024 points, K=9 kernel offsets, C=32 channels):
  y[i,c] = leaky( sum_{n,k} exp(-|pi - pn - kk|^2/2) * (w @ conv_w[k])[n,c] )
  out = BN(y)+w -> MLP(32->128->32, leaky, BN) residual.

Factorization (k-independent Gaussian Gram matrix):
  exp(-|pi - pn - kk|^2/2) = G[n,i] * A[i,k] * B[n,k]
  G[n,i] = exp(pn.pi - |pn|^2/2 - |pi|^2/2)
  A[i,k] = exp(pi.kk),  B[n,k] = exp(-pn.kk - |kk|^2/2)   (host-precomputed)
So y[i,c] = sum_k A[i,k] * (G.T @ (B[:,k] * cw[:,k,:]))[i,c].

v2 performance structure:
  - G's pn.pi matmul runs in bf16 with a hi/lo split of the coordinates
    (contract dim 8) -> full tensor-engine rate with ~fp32 exponent accuracy.
  - Phase B / phase D matmuls in bf16 (1 cyc/row); MLP matmuls in fp32r.
  - B and A factors are host-replicated across the 32 channels so the
    (n,k)- and (i,k)-scalings are a few big vector ops, not 100+ tiny ones.
  - ONE AllGather ships each core's yT slice + BN0 partial sums; every core
    then redundantly computes full-batch h (for BN1 stats) and its own
    output slice fully locally (no second collective, no rank-dependent
    indexing).
  - BN rsqrt via DVE Newton iteration (quake trick) -> no Sqrt tables on
    the scalar engine; ACT only ever loads Exp and Lrelu tables.
  - Optional tiny dummy AllGather at kernel start absorbs first-collective
    warmup / rank-start skew concurrently with compute.

Sharding: 8 cores = 4 graphs x 2 halves of the 1024 output rows. Each core
computes its [512, 32] conv-output slice and emits out[32, 512]; the host
concatenates.

Self-contained: hardcodes B=4, N=1024, K=9, C=32, CM=128, 8 cores.
"""

import numpy as np
import ml_dtypes

import concourse.bass as bass
import concourse.bacc as bacc
import concourse.mybir as mybir
import concourse.tile as tile
from concourse import masks
from concourse.bass_utils import run_bass_kernel_spmd

F32 = mybir.dt.float32
F32R = mybir.dt.float32r
BF16 = mybir.dt.bfloat16
I32 = mybir.dt.int32
AF = mybir.ActivationFunctionType
ALU = mybir.AluOpType
AX = mybir.AxisListType

NB, N, K, C, CM = 4, 1024, 9, 32, 128
NCORES = 8
EPS = 1e-5
SLOPE = 0.01
NT = NB * N
KC = K * C  # 288
MAGIC = 0x5F3759DF

WARM_CC = True  # early dummy collective to absorb first-CC warmup


def _r(ap):
    return ap.bitcast(F32R)


def _build_module():
    nc = bacc.Bacc("TRN2", target_bir_lowering=False, debug=False,
                   num_devices=NCORES)

    def din(name, shape, dt=F32):
        return nc.dram_tensor(name, list(shape), dt, kind="ExternalInput").ap()

    # packed inputs (few big DMAs): see _host_prep for layouts
    packp_d = din("packp", (8, 1536), BF16)   # pn8 | pi8 hi/lo coord rows
    packw_d = din("packw", (C, 6048), BF16)   # wTb|wtext|wtob|w1b16|cwall
    packf_d = din("packf", (128, 3456), BF16)  # bexp | aexp (c-replicated)
    packs_d = din("packs", (C + 1, 644))      # w1e | g0,be0,b2 | wto
    p128_d = din("p128", (128, 44))           # negsqn(8)|w2(32)|g1|be1|pad

    out_d = nc.dram_tensor("out", [C, 512], F32, kind="ExternalOutput").ap()
    dbg_y_d = nc.dram_tensor("dbg_y", [C, 516], BF16,
                             kind="ExternalOutput").ap()
    dbg_s_d = nc.dram_tensor("dbg_s", [128, 8], F32,
                             kind="ExternalOutput").ap()

    with tile.TileContext(nc) as tc:
        with (
            tc.tile_pool(name="const", bufs=1) as pc,
            tc.tile_pool(name="big", bufs=1) as pb,
            tc.tile_pool(name="work", bufs=3) as pw,
            tc.tile_pool(name="dram", bufs=1, space="DRAM") as pd,
        ):
            # ---- optional collective warmup (content irrelevant) ----
            if WARM_CC:
                dmy_in = pd.tile([1, 8], F32, tag="dmy_in")
                dmy_out = pd.tile([NCORES, 8], F32, tag="dmy_out")
                nc.gpsimd.collective_compute(
                    "AllGather", ALU.bypass,
                    replica_groups=[list(range(NCORES))],
                    ins=[dmy_in[:].opt()], outs=[dmy_out[:].opt()])

            # ---- input loads ----
            def load(name, ap, shape, dt=F32, pool=pc):
                t = pool.tile(list(shape), dt, tag=name, name=name)
                nc.sync.dma_start(out=t[:], in_=ap)
                return t

            p128 = load("p128", p128_d, (128, 44))
            packp = load("packp", packp_d, (8, 1536), BF16)
            packw = load("packw", packw_d, (C, 6048), BF16, pool=pb)
            packf = load("packf", packf_d, (128, 3456), BF16, pool=pb)
            packs = load("packs", packs_d, (C + 1, 644))

            pn8 = packp[:, 0:N]
            pi8 = packp[:, N:N + 512]
            wTb = packw[:, 0:1024]
            wtext = packw[:, 1024:5120]
            wtob = packw[:, 5120:5632]
            w1b16 = packw[:, 5632:5760]
            cwall = packw[:, 5760:6048]
            bexp = packf[:, 0:8 * KC]
            aexp = packf[:, 8 * KC:12 * KC]
            w1e = packs[:, 0:CM]
            g0c = packs[0:C, 128:129]
            be0c = packs[0:C, 129:130]
            b2c = packs[0:C, 130:131]
            wto = packs[0:C, 131:643]

            negsqn = p128[:, 0:8]
            w2 = p128[:, 8:40]
            g1c = p128[:, 40:41]
            be1c = p128[:, 41:42]

            ident = pc.tile([128, 128], F32, tag="ident")
            masks.make_identity(nc, ident[:])
            warm0 = pc.tile([128, 1], F32, tag="warm0")
            nc.scalar.activation(warm0[:], ident[:, 0:1], AF.Exp)

            ag_y = pb.tile([C, 516], BF16, tag="ag_y")
            ysum_p = pc.tile([C, 4], F32, tag="ysum")
            ysq_p = pc.tile([C, 4], F32, tag="ysq")

            with (
                tc.tile_pool(name="psG", bufs=2, space="PSUM") as psG,
                tc.tile_pool(name="psB", bufs=2, space="PSUM") as psB,
                tc.tile_pool(name="psD", bufs=2, space="PSUM") as psD,
                tc.tile_pool(name="psC", bufs=1, space="PSUM") as psC,
            ):
                # ---- G[n,i] = exp(pn.pi - |pn|^2/2 - |pi|^2/2), bf16 hi/lo
                g_sb = [pb.tile([128, 512], BF16, tag=f"g{j}", name=f"g{j}")
                        for j in range(8)]
                # ---- cw'[n,(k,c)] = B[n,k] * (w @ conv_w[k]), bf16
                cw_sb = [pb.tile([128, KC], BF16, tag=f"cw{j}", name=f"cw{j}")
                         for j in range(8)]
                for j in range(8):
                    psg = psG.tile([128, 512], F32, tag="g")
                    nc.tensor.matmul(psg[:], lhsT=pn8[:, j * 128:(j + 1) * 128],
                                     rhs=pi8, start=True, stop=True)
                    nc.scalar.activation(g_sb[j][:], psg[:], AF.Exp,
                                         bias=negsqn[:, j:j + 1], scale=1.0)
                    psb = psB.tile([128, KC], F32, tag="b")
                    nc.tensor.matmul(psb[:], lhsT=wTb[:, j * 128:(j + 1) * 128],
                                     rhs=cwall, start=True, stop=True)
                    nc.vector.tensor_tensor(
                        cw_sb[j][:], psb[:],
                        bexp[:, j * KC:(j + 1) * KC], op=ALU.mult)

                # ---- main contraction + combine, two t's at a time ----
                # (2 PSUM accumulator banks; pair 0-1 combines while pair
                # 2-3 accumulates)
                for tp in range(2):
                    pys = [psD.tile([128, KC], F32, tag="py",
                                    name=f"py{tp}_{ti}")
                           for ti in range(2)]
                    for j in range(8):
                        for ti in range(2):
                            t = tp * 2 + ti
                            nc.tensor.matmul(
                                pys[ti][:],
                                lhsT=g_sb[j][:, t * 128:(t + 1) * 128],
                                rhs=cw_sb[j][:],
                                start=(j == 0), stop=(j == 7))
                    for ti in range(2):
                        t = tp * 2 + ti
                        ya = pw.tile([128, KC], F32, tag="ya")
                        nc.vector.tensor_tensor(
                            ya[:], pys[ti][:], aexp[:, t * KC:(t + 1) * KC],
                            op=ALU.mult)
                        y_t = pw.tile([128, C], F32, tag="yt")
                        nc.vector.tensor_reduce(
                            y_t[:], ya[:].rearrange("p (k c) -> p c k", k=K),
                            axis=AX.X, op=ALU.add)
                        y_l = pw.tile([128, C], F32, tag="yl")
                        nc.vector.scalar_tensor_tensor(
                            y_l[:], y_t[:], SLOPE, y_t[:],
                            op0=ALU.mult, op1=ALU.max)
                        ptr = psC.tile([C, 128], F32, tag="tr")
                        nc.tensor.transpose(ptr[:], y_l[:], ident[:])
                        nc.vector.tensor_scalar(
                            ag_y[:, t * 128:(t + 1) * 128], ptr[:], 0.0, 0.0,
                            op0=ALU.add, op1=ALU.add,
                            accum_out=ysum_p[:, t:t + 1])
                        agt = ag_y[:, t * 128:(t + 1) * 128]
                        sq = pw.tile([C, 128], BF16, tag="sq")
                        nc.vector.scalar_tensor_tensor(
                            sq[:], agt, 1.0, agt, op0=ALU.mult, op1=ALU.mult,
                            accum_out=ysq_p[:, t:t + 1])
                stat_cols = ag_y[:, 512:516].bitcast(F32)
                nc.vector.tensor_reduce(stat_cols[:, 0:1], ysum_p[:],
                                        axis=AX.X, op=ALU.add)
                nc.vector.tensor_reduce(stat_cols[:, 1:2], ysq_p[:],
                                        axis=AX.X, op=ALU.add)

            # ---- the one AllGather: own yT slice + BN0 partial sums ----
            ag_in = pd.tile([C, 516], BF16, tag="ag_in")
            ag_out = pd.tile([C * NCORES, 516], BF16, tag="ag_out")
            nc.sync.dma_start(out=ag_in[:], in_=ag_y[:])
            nc.gpsimd.collective_compute(
                "AllGather", ALU.bypass,
                replica_groups=[list(range(NCORES))],
                ins=[ag_in[:].opt()], outs=[ag_out[:].opt()])

            with (
                tc.tile_pool(name="psH", bufs=1, space="PSUM") as psH,
                tc.tile_pool(name="psR", bufs=2, space="PSUM") as psR,
                tc.tile_pool(name="psS", bufs=1, space="PSUM") as psS,
                tc.tile_pool(name="psT", bufs=1, space="PSUM") as psT,
            ):
                # ---- during the collective: w-side of MLP1 for chunks 0-2,
                # Lrelu table preload dummy
                NHOLD = 3
                hold = [psH.tile([128, 512], F32, tag=f"h{ci}", name=f"h{ci}")
                        for ci in range(NHOLD)]
                for ci in range(NHOLD):
                    nc.tensor.matmul(hold[ci][:], lhsT=w1b16,
                                     rhs=wtext[:, ci * 512:(ci + 1) * 512],
                                     start=True, stop=False)
                # preload the Sqrt table while the collective runs
                warm = pw.tile([128, 1], F32, tag="warm")
                nc.scalar.activation(warm[:], p128[:, 42:43], AF.Sqrt)

                # ---- receive the gather ----
                agv = ag_out[:].rearrange("(jj p) n -> p jj n", jj=NCORES)
                statj = pc.tile([C, 32], BF16, tag="statj")
                nc.sync.dma_start(
                    out=statj[:].rearrange("p (jj s) -> p jj s", s=4),
                    in_=agv[:, :, 512:516])
                yt_full = pb.tile([C, NT], BF16, tag="yt_full")
                nc.sync.dma_start(
                    out=yt_full[:, 0:2048].rearrange("p (jj n) -> p jj n",
                                                     jj=4),
                    in_=agv[:, 0:4, 0:512])
                nc.sync.dma_start(
                    out=yt_full[:, 2048:4096].rearrange("p (jj n) -> p jj n",
                                                        jj=4),
                    in_=agv[:, 4:8, 0:512])

                # ---- BN0 global stats ----
                statv = statj[:].bitcast(F32).rearrange("p (jj s) -> p s jj", s=2)
                tot = pc.tile([C, 2], F32, tag="tot")
                nc.vector.tensor_reduce(tot[:, 0:1], statv[:, 0:1, :],
                                        axis=AX.X, op=ALU.add)
                nc.vector.tensor_reduce(tot[:, 1:2], statv[:, 1:2, :],
                                        axis=AX.X, op=ALU.add)
                mom = pc.tile([C, 2], F32, tag="mom")
                nc.vector.tensor_scalar_mul(mom[:], tot[:], 1.0 / NT)
                var0 = pc.tile([C, 1], F32, tag="var0")
                nc.vector.tensor_tensor(var0[:], mom[:, 0:1], mom[:, 0:1],
                                        op=ALU.mult)
                nc.vector.scalar_tensor_tensor(
                    var0[:], mom[:, 1:2], EPS, var0[:],
                    op0=ALU.add, op1=ALU.subtract)

                def rsqrt(dst, src, p, pref):
                    # ACT Sqrt (table preloaded during the AG) + DVE recip
                    st = pc.tile([p, 1], F32, tag=pref + "s")
                    nc.scalar.activation(st[:], src[:], AF.Sqrt)
                    nc.vector.reciprocal(dst[:], st[:])

                rstd0 = pc.tile([C, 1], F32, tag="rstd0")
                rsqrt(rstd0, var0, C, "r0")
                scale0 = pc.tile([C, 1], F32, tag="scale0")
                nc.vector.tensor_tensor(scale0[:], rstd0[:], g0c,
                                        op=ALU.mult)
                nscale0 = pc.tile([C, 1], F32, tag="nscale0")
                nc.vector.tensor_scalar_mul(nscale0[:], scale0[:], -1.0)
                shift0 = pc.tile([C, 1], F32, tag="shift0")
                nc.vector.scalar_tensor_tensor(
                    shift0[:], mom[:, 0:1], nscale0[:], be0c,
                    op0=ALU.mult, op1=ALU.add)

                # bias1 = w1.T @ shift0 + b1  (ones-row trick via w1e row C)
                sh1e = pc.tile([C + 1, 1], F32, tag="sh1e")
                nc.vector.memset(sh1e[C:C + 1, :], 1.0)
                nc.vector.tensor_copy(out=sh1e[0:C, :], in_=shift0[:])
                psb1 = psT.tile([CM, 1], F32, tag="b1")
                nc.tensor.matmul(psb1[:], lhsT=w1e, rhs=sh1e[:],
                                 start=True, stop=True)
                bias1 = pc.tile([CM, 1], F32, tag="bias1")
                nc.vector.tensor_copy(out=bias1[:], in_=psb1[:])
                w1p = pc.tile([C, CM], BF16, tag="w1p")
                nc.vector.tensor_scalar(w1p[:], w1e[0:C, :], scale0[:], None,
                                        op0=ALU.mult)

                # ---- full-batch h chunks: leaky(w1'.T yT + w1.T wT + bias1)
                hsum_p = pc.tile([CM, 8], F32, tag="hsum")
                hsq_p = pc.tile([CM, 8], F32, tag="hsq")
                for ci in range(8):
                    if ci < NHOLD:
                        ps = hold[ci]
                    else:
                        ps = psR.tile([128, 512], F32, tag="hr")
                        nc.tensor.matmul(
                            ps[:], lhsT=w1b16,
                            rhs=wtext[:, ci * 512:(ci + 1) * 512],
                            start=True, stop=False)
                    nc.tensor.matmul(
                        ps[:], lhsT=w1p[:],
                        rhs=yt_full[:, ci * 512:(ci + 1) * 512],
                        start=False, stop=True)
                    hcs = pw.tile([CM, 512], BF16, tag="hcs")
                    nc.scalar.activation(hcs[:], ps[:], AF.Lrelu,
                                         bias=bias1[:], scale=1.0, alpha=SLOPE,
                                         accum_out=hsum_p[:, ci:ci + 1])
                    sqh = pw.tile([CM, 512], BF16, tag="sqh")
                    nc.vector.scalar_tensor_tensor(
                        sqh[:], hcs[:], 1.0, hcs[:],
                        op0=ALU.mult, op1=ALU.mult,
                        accum_out=hsq_p[:, ci:ci + 1])

                # ---- own-slice yres + h (fully local, no rank indexing) ----
                yres = pb.tile([C, 512], F32, tag="yres")
                nc.vector.tensor_scalar(yres[:], ag_y[:, 0:512], scale0[:],
                                        shift0[:], op0=ALU.mult, op1=ALU.add)
                nc.vector.tensor_tensor(yres[:], yres[:], wto, op=ALU.add)
                pso = psR.tile([128, 512], F32, tag="hr")
                nc.tensor.matmul(pso[:], lhsT=w1b16, rhs=wtob,
                                 start=True, stop=False)
                nc.tensor.matmul(pso[:], lhsT=w1p[:],
                                 rhs=ag_y[:, 0:512],
                                 start=False, stop=True)
                h_own = pb.tile([CM, 512], BF16, tag="h_own")
                nc.scalar.activation(h_own[:], pso[:], AF.Lrelu,
                                     bias=bias1[:], scale=1.0, alpha=SLOPE)

                # ---- BN1 stats + fold into w2 ----
                tot1 = pc.tile([CM, 2], F32, tag="tot1")
                nc.vector.tensor_reduce(tot1[:, 0:1], hsum_p[:], axis=AX.X,
                                        op=ALU.add)
                nc.vector.tensor_reduce(tot1[:, 1:2], hsq_p[:], axis=AX.X,
                                        op=ALU.add)
                mom1 = pc.tile([CM, 2], F32, tag="mom1")
                nc.vector.tensor_scalar_mul(mom1[:], tot1[:], 1.0 / NT)
                var1 = pc.tile([CM, 1], F32, tag="var1")
                nc.vector.tensor_tensor(var1[:], mom1[:, 0:1], mom1[:, 0:1],
                                        op=ALU.mult)
                nc.vector.scalar_tensor_tensor(
                    var1[:], mom1[:, 1:2], EPS, var1[:],
                    op0=ALU.add, op1=ALU.subtract)
                rstd1 = pc.tile([CM, 1], F32, tag="rstd1")
                rsqrt(rstd1, var1, CM, "r1")
                scale1 = pc.tile([CM, 1], F32, tag="scale1")
                nc.vector.tensor_tensor(scale1[:], rstd1[:], g1c,
                                        op=ALU.mult)
                nscale1 = pc.tile([CM, 1], F32, tag="nscale1")
                nc.vector.tensor_scalar_mul(nscale1[:], scale1[:], -1.0)
                shift1 = pc.tile([CM, 1], F32, tag="shift1")
                nc.vector.scalar_tensor_tensor(
                    shift1[:], mom1[:, 0:1], nscale1[:], be1c,
                    op0=ALU.mult, op1=ALU.add)
                w2p = pc.tile([CM, C], BF16, tag="w2p")
                nc.vector.tensor_scalar(w2p[:], w2, scale1[:], None,
                                        op0=ALU.mult)
                psb2 = psT.tile([C, 1], F32, tag="b2")
                nc.tensor.matmul(psb2[:], lhsT=w2, rhs=shift1[:],
                                 start=True, stop=True)
                bias2 = pc.tile([C, 1], F32, tag="bias2")
                nc.vector.tensor_tensor(bias2[:], psb2[:], b2c, op=ALU.add)

                # ---- delta = w2'.T @ h_own + bias2; out = yres + delta ----
                psd = psS.tile([C, 512], F32, tag="delta")
                nc.tensor.matmul(psd[:], lhsT=w2p[:], rhs=h_own[:],
                                 start=True, stop=True)
                out_sb = pw.tile([C, 512], F32, tag="outsb")
                nc.vector.scalar_tensor_tensor(
                    out_sb[:], psd[:], bias2[:], yres[:],
                    op0=ALU.add, op1=ALU.add)
                nc.sync.dma_start(out=out_d, in_=out_sb[:])

                # ---- debug dumps ----
                nc.sync.dma_start(out=dbg_y_d, in_=ag_y[:])
                dbg_s = pc.tile([128, 8], F32, tag="dbg_s")
                nc.vector.memset(dbg_s[:], 0.0)
                nc.vector.tensor_copy(out=dbg_s[0:C, 0:2], in_=tot[:])
                nc.vector.tensor_copy(out=dbg_s[0:C, 2:3], in_=var0[:])
                nc.vector.tensor_copy(out=dbg_s[0:C, 3:4], in_=rstd0[:])
                nc.vector.tensor_copy(out=dbg_s[0:C, 4:5], in_=scale0[:])
                nc.vector.tensor_copy(out=dbg_s[0:C, 5:6], in_=shift0[:])
                nc.vector.tensor_copy(out=dbg_s[:, 6:7], in_=bias1[:])
                nc.vector.tensor_copy(out=dbg_s[:, 7:8], in_=rstd1[:])
                nc.sync.dma_start(out=dbg_s_d, in_=dbg_s[:])

    nc.compile()
    return nc


_NC_CACHE = {}


def _get_module():
    if "nc" not in _NC_CACHE:
        _NC_CACHE["nc"] = _build_module()
    return _NC_CACHE["nc"]


BF = ml_dtypes.bfloat16


def _split_hi_lo(x):
    hi = x.astype(BF)
    lo = (x - hi.astype(np.float32)).astype(BF)
    return hi, lo


def _host_prep(inputs):
    pos = np.asarray(inputs["positions"], np.float32)
    w = np.asarray(inputs["weights"], np.float32)
    kp = np.asarray(inputs["kernel_pos"], np.float32)
    cw = np.asarray(inputs["conv_w"], np.float32)
    posb = pos.reshape(NB, N, 2)
    wb = w.reshape(NB, N, C)
    kk2 = 0.5 * (kp ** 2).sum(1)                       # [9]
    cwall = cw.transpose(1, 0, 2).reshape(C, KC).astype(BF)
    wTfull = np.ascontiguousarray(w.T)                 # [32, 4096]
    w1 = np.asarray(inputs["w1"], np.float32)
    w2 = np.asarray(inputs["w2"], np.float32)
    w1e = np.concatenate(
        [w1, np.asarray(inputs["b1"], np.float32).reshape(1, CM)], axis=0)
    p128 = np.zeros((128, 44), np.float32)
    p128[:, 8:40] = w2
    p128[:, 40] = np.asarray(inputs["bn1_gamma"], np.float32)
    p128[:, 41] = np.asarray(inputs["bn1_beta"], np.float32)
    p128[:, 42] = np.int32(1).view(np.float32)
    p128[:, 43] = np.int32(MAGIC).view(np.float32)

    in_maps = []
    for j in range(NCORES):
        b, off = j // 2, (j % 2) * 512
        p = posb[b]
        pi = p[off:off + 512]
        xh, xl = _split_hi_lo(p[:, 0])
        yh, yl = _split_hi_lo(p[:, 1])
        one = np.ones(N, BF)
        pn8 = np.stack([xh, xh, xl, yh, yh, yl, one, one])
        bias = -0.5 * (pi ** 2).sum(1)
        bh, bl = _split_hi_lo(bias)
        xih, xil = _split_hi_lo(pi[:, 0])
        yih, yil = _split_hi_lo(pi[:, 1])
        pi8 = np.stack([xih, xil, xih, yih, yil, yih, bh, bl])
        packp = np.concatenate([pn8, pi8], axis=1)          # [8, 1536]

        packw = np.empty((C, 6048), BF)
        packw[:, 0:1024] = wb[b].T.astype(BF)
        packw[:, 1024:5120] = wTfull.astype(BF)
        packw[:, 5120:5632] = wTfull[:, j * 512:(j + 1) * 512].astype(BF)
        packw[:, 5632:5760] = w1.astype(BF)
        packw[:, 5760:6048] = cwall

        p128j = p128.copy()
        p128j[:, 0:8] = (-0.5 * (p ** 2).sum(1)).reshape(8, 128).T
        dotn = (p @ kp.T).astype(np.float32)            # [1024, 9]
        Bmat = np.exp(-dotn - kk2[None, :]).astype(np.float32)
        Amat = np.exp((pi @ kp.T).astype(np.float32)).astype(np.float32)
        packf = np.empty((128, 3456), BF)
        packf[:, 0:8 * KC] = np.broadcast_to(
            Bmat.reshape(8, 128, K, 1).transpose(1, 0, 2, 3),
            (128, 8, K, C)).reshape(128, 8 * KC).astype(BF)
        packf[:, 8 * KC:] = np.broadcast_to(
            Amat.reshape(4, 128, K, 1).transpose(1, 0, 2, 3),
            (128, 4, K, C)).reshape(128, 4 * KC).astype(BF)

        packs = np.zeros((C + 1, 644), np.float32)
        packs[:, 0:CM] = w1e
        packs[0:C, 128] = np.asarray(inputs["bn_gamma"], np.float32)
        packs[0:C, 129] = np.asarray(inputs["bn_beta"], np.float32)
        packs[0:C, 130] = np.asarray(inputs["b2"], np.float32)
        packs[0:C, 131:643] = wTfull[:, j * 512:(j + 1) * 512]

        in_maps.append(dict(
            packp=np.ascontiguousarray(packp),
            packw=np.ascontiguousarray(packw),
            packf=np.ascontiguousarray(packf),
            packs=np.ascontiguousarray(packs),
            p128=np.ascontiguousarray(p128j)))
    return in_maps


def _run(inputs, trace=False):
    nc = _get_module()
    in_maps = _host_prep(inputs)
    res = run_bass_kernel_spmd(nc, in_maps, core_ids=list(range(NCORES)),
                               trace=trace)
    out = np.concatenate([np.asarray(res.results[j]["out"])
                          for j in range(NCORES)], axis=1)   # [32, 4096]
    return np.ascontiguousarray(out.T), res


def kernel(**inputs):
    out, _ = _run(inputs, trace=False)
    return out
